# revision 28
# baseline (speedup 1.0000x reference)
"""Trainium2 Bass kernel for nn_MinLoss_12343736009330.

Math: the reference loss is
    loss = sum_{b,s} || pf[b,s] - gf[b,match[b,s]] ||_2
where pf/gf are the per-(batch, source) flattened [L=T*D] signals, and match
is a greedy assignment on the 4x4 Euclidean cdist.  Since
    ||pf[s] - gf[m]||^2 = pn[s] + gn[m] - 2 <pf[s], gf[m]>,
the whole computation reduces to the per-batch 8x8 Gram matrix of the
8 vectors {pf[0..4], gf[0..4]} plus a tiny 4x4 greedy matching.

Sharding: batch axis (16) across 8 cores -> 2 batches/core.

v2 (default "fp8safe", ~41us vs the 69-72us fp32/bf16 baseline): inputs
are quantized to fp8 e4m3 on the host (measured loss error ~4e-4 vs the
2e-2 gate), quartering HBM traffic vs fp32.  ~22 junk matmuls on a
memset tile pre-ramp the PE p-state while the first chunk is in flight;
each tile then streams in column chunks alternating across the sync and
gpsimd DMA rings while plain fp8 matmuls (1 col/cycle, same rate as
bf16 but 1/4 the bytes) accumulate the interleaved Gram per batch.
Selector matmuls reduce the 8x8 diagonal blocks, one DVE op + one
matmul flatten the Gram to a single partition with 0.5 pre-scaled
norms, so s = 0.5*d^2 and the greedy matching runs on s using row/col
penalty vectors (cheaper than the 16x16 penalty-table form).  Per-batch
structure hides batch 0's reduction and matching under batch 1's DMA.
Host applies sqrt(2s) and sums across cores.

Measured on this setup: per-core DMA is throttled to ~280 GB/s
aggregate no matter how many rings are used; PE fp8 matmuls run ~56ns
per [128,128] instruction (DoubleRow perf mode executes but is NOT
faster on real HW: ~73ns); the fused DVE ops (tensor_tensor_reduce /
scalar_tensor_tensor) crashed NEFF execution and are kept only in the
non-default variants; the measured window carries a fixed ~7us
semaphore-teardown epilogue and ~3us of DMA spin-up that no kernel
structure avoids.
"""

import os
import sys

import numpy as np
import ml_dtypes

try:
    import concourse.bass as bass  # noqa: F401
except ImportError:
    sys.path.insert(0, "/opt/trn_rl_repo")

import concourse.bass as bass  # noqa: F811
import concourse.tile as tile
from concourse import bacc, mybir
from concourse.bass_utils import run_bass_kernel_spmd


def _install_ntff_hook_shim():
    """The bare agent image lacks ``antenv.axon_hooks``, so trace=True under
    axon would ImportError.  Recreate the module with the ctypes-based NTFF
    hook from trn_agent_boot (degrades to hook=None if unavailable)."""
    import types

    try:
        import antenv.axon_hooks  # noqa: F401

        return
    except ImportError:
        pass
    hook = None
    try:
        from trn_agent_boot.trn_boot import _ntff_profile_via_ctypes

        so_path = "/opt/axon/libaxon_pjrt.so"
        if os.path.exists(so_path):
            hook = _ntff_profile_via_ctypes(so_path)
    except Exception:
        hook = None
    import antenv

    mod = types.ModuleType("antenv.axon_hooks")
    mod.get_axon_ntff_profile_hook = lambda: hook  # type: ignore[attr-defined]

    def _set(h):
        nonlocal hook
        hook = h

    mod.set_axon_ntff_profile_hook = _set  # type: ignore[attr-defined]
    sys.modules["antenv.axon_hooks"] = mod
    antenv.axon_hooks = mod


_install_ntff_hook_shim()

F32 = mybir.dt.float32
FP8 = mybir.dt.float8e4
NP_FP8 = ml_dtypes.float8_e4m3

S, T, B, D = 4, 512, 16, 512
N_CORES = 8
NB = B // N_CORES          # batches per core
BIG = 1.0e30
FMAX = 3.0e38

# "fp8dr":   DoubleRow fp8 matmuls (2 cols/cycle), [64,64] Gram PSUM.
# "fp8":     plain fp8 matmuls, [128,128] Gram PSUM (fallback).
# "fp8drh":  fp8dr but the device returns the per-batch 8x8 Grams and the
#            4x4 greedy matching runs on the host with the final reduction.
# "fp8safe": plain fp8 matmuls + baseline-style DVE ops only (no fused
#            reduce ops, no Activation-ring DMA).
VARIANT = os.environ.get("MINLOSS_V", "fp8safe")


def _build_consts() -> np.ndarray:
    """Host-side constant block, DMA'd once: [128, 512] fp32.

    row 0, cols 0:256: penalty table TBL[j*16+k] = BIG if entries j and k
    of the flattened 4x4 dist matrix share a row or column.
    cols 256:384: 128x128 identity (diagonal-block selector matmuls; the
    fp8dr variant uses the top-left 64x64 of it).
    rows 0:8, cols 384:448: flatten mask M[j, 8i+k] = (i==j)*(0.5 if k==j
    else 1) -- the 0.5 pre-halves the norms so s = 0.5*d^2 comes out of
    the flatten matmul directly.
    rows 0:8, col 448: ones (flatten matmul stationary).
    """
    c = np.zeros((128, 512), np.float32)
    idx = np.arange(256)
    jj, kk = idx // 16, idx % 16
    c[0, 0:256] = np.where((jj // 4 == kk // 4) | (jj % 4 == kk % 4), BIG, 0.0)
    c[:, 256:384] = np.eye(128, dtype=np.float32)
    m = np.zeros((8, 8, 8), np.float32)
    for j in range(8):
        m[j, j, :] = 1.0
        m[j, j, j] = 0.5
    c[0:8, 384:448] = m.reshape(8, 64)
    c[0:8, 448] = 1.0
    return c


CONSTS = _build_consts()


def build_nc(variant: str = VARIANT):
    nc = bacc.Bacc(
        "TRN2",
        target_bir_lowering=False,
        debug=False,
        enable_asserts=True,
        num_devices=N_CORES,
    )
    if variant in ("fp8dr", "fp8drh", "fp8drsafe"):
        # xa[b, tb, p, i*4096 + g*64 + id*8 + v] = vec v at
        # t = 256*tb + 128*i + p, d = 8*g + id  (v 0..3 preds, 4..7 gts)
        xa_t = nc.dram_tensor("xa", [NB, 2, 128, 8192], FP8, kind="ExternalInput").ap()
    else:
        # xa[b, tb, p, g*128 + i16*8 + v] = vec v at t = 128*tb + p,
        # d = 16*g + i16
        xa_t = nc.dram_tensor("xa", [NB, 4, 128, 4096], FP8, kind="ExternalInput").ap()
    consts_t = nc.dram_tensor("consts", [128, 512], F32, kind="ExternalInput").ap()
    # the 8 greedy minima (0.5 * squared distances); host: sqrt(2x) + sum.
    # fp8drh instead returns the per-batch 8x8 Grams in "loss" [NB*8, 8].
    oshape = [NB * 8, 8] if variant == "fp8drh" else [1, 2 * S]
    loss_t = nc.dram_tensor("loss", oshape, F32, kind="ExternalOutput").ap()

    with tile.TileContext(nc) as tc:
        _build_tile(tc, xa_t, consts_t, loss_t, variant)

    nc.compile()
    return nc


def _build_tile(tc, xa_t, consts_t, loss_t, variant):
    nc = tc.nc
    import contextlib

    dr = variant in ("fp8dr", "fp8drh", "fp8drsafe")
    host_match = variant == "fp8drh"
    safe = variant in ("fp8safe", "fp8drsafe")  # baseline-proven op classes only
    ntiles = 2 if dr else 4            # DMA tiles per batch
    ngr = 64 if dr else 32             # matmul groups per tile
    cdim = 64 if dr else 128           # Gram PSUM dim
    nq = cdim // 8                     # selector matmuls per Gram

    ctx = contextlib.ExitStack()
    with ctx:
        b_pool = ctx.enter_context(tc.tile_pool(name="b", bufs=NB * ntiles))
        psum_pool = ctx.enter_context(tc.tile_pool(name="psum", bufs=1, space="PSUM"))
        psumf_pool = ctx.enter_context(tc.tile_pool(name="psumf", bufs=1, space="PSUM"))
        consts_pool = ctx.enter_context(tc.tile_pool(name="consts", bufs=1))
        small_pool = ctx.enter_context(tc.tile_pool(name="small", bufs=1))

        csb = consts_pool.tile([128, 512], F32)
        # consts ride the late-starting gpsimd ring, which also carries the
        # smallest data share
        nc.gpsimd.dma_start(out=csb[:, :], in_=consts_t[:, :])
        tbl16 = csb[0:1, 0:256].rearrange("p (j k) -> p j k", k=16)
        ident = csb[:, 256:384]
        mask8s = csb[0:8, 384:448].rearrange("p (i k) -> p i k", k=8)
        ones8 = csb[0:8, 448:449]

        # the 8 greedy minima (0.5 * squared dists)
        loss4 = small_pool.tile([1, 2 * S], F32, tag="loss4")

        if safe:
            rings = [nc.sync, nc.scalar, nc.gpsimd]
        else:
            rings = [nc.sync, nc.scalar]
        ring_cnt = [0]

        def next_ring():
            r = rings[ring_cnt[0] % len(rings)]
            ring_cnt[0] += 1
            return r

        def chunk_ring(ib, tb, ch, nch):
            return next_ring()

        psum_as = [
            psum_pool.tile([cdim, cdim], F32, tag=f"pA{i}", name=f"psum_a{i}")
            for i in range(NB)
        ]

        # Junk matmuls on a memset tile ramp the PE p-state (0.65 -> 2.4
        # GHz needs a few us of continuous execution) while the first data
        # chunk is still in flight.
        nwarm = int(os.environ.get("MINLOSS_WARM", "22"))
        if nwarm:
            warm_ps = psum_pool.tile([cdim, cdim], F32, tag="warmps")
            wsb = small_pool.tile([128, 128], FP8, tag="warm")
            nc.vector.memset(wsb[:, :], 1.0)
            for _ in range(nwarm):
                nc.tensor.matmul(
                    warm_ps[:, :],
                    lhsT=wsb[:, 0:cdim],
                    rhs=wsb[:, 0:cdim],
                    start=True,
                    stop=True,
                )

        psgs = [psumf_pool.tile([8, 8], F32, tag=f"psg{i}", name=f"psg{i}") for i in range(NB)]
        c_sbs = [
            small_pool.tile([cdim, cdim], F32, tag=f"c{i}", name=f"c_sb{i}")
            for i in range(NB)
        ]
        acc8s = [
            small_pool.tile([8, 8], F32, tag=f"acc{i}", name=f"acc8{i}")
            for i in range(NB)
        ]

        def emit_tile(ib, tb):
            psum_a = psum_as[ib]
            if True:
                last_tile = tb == ntiles - 1
                psum = psum_a
                if dr:
                    xtile = b_pool.tile([128, 8192], FP8)
                    xv = xtile[:, :].rearrange("p (i c) -> p i c", i=2)
                    iv = xa_t[ib, tb, :, :].rearrange("p (i c) -> p i c", i=2)
                else:
                    xtile = b_pool.tile([128, 4096], FP8)
                    xv = xtile[:, :]
                    iv = xa_t[ib, tb, :, :]

                def emit_mm(g, psum=psum, xv=xv, tb=tb, last_tile=last_tile):
                    first = tb == 0 and g == 0
                    last = last_tile and g == ngr - 1
                    if dr:
                        op = xv[:, :, g * 64 : (g + 1) * 64]
                        nc.tensor.matmul(
                            psum[:, :],
                            lhsT=op,
                            rhs=op,
                            start=first,
                            stop=last,
                            perf_mode=mybir.MatmulPerfMode.DoubleRow,
                        )
                    else:
                        op = xv[:, g * 128 : (g + 1) * 128]
                        nc.tensor.matmul(
                            psum[:, :], lhsT=op, rhs=op, start=first, stop=last
                        )

                if True:
                    # column chunks: matmuls on chunk k stream while chunk
                    # k+1 is still in flight; finer chunks on the last tile
                    # so its matmuls overlap the DMA tail
                    nch = 4 if (last_tile or (ib == 0 and tb == 0)) else 2
                    gpc = ngr // nch
                    for ch in range(nch):
                        g0, g1 = ch * gpc, (ch + 1) * gpc
                        if dr:
                            osl = xv[:, :, g0 * 64 : g1 * 64]
                            isl = iv[:, :, g0 * 64 : g1 * 64]
                        else:
                            osl = xv[:, g0 * 128 : g1 * 128]
                            isl = iv[:, g0 * 128 : g1 * 128]
                        chunk_ring(ib, tb, ch, nch).dma_start(out=osl, in_=isl)
                        for g in range(g0, g1):
                            emit_mm(g)

        def emit_phase2(ib):
            psum_a = psum_as[ib]
            psg = psgs[ib]
            c_sb = c_sbs[ib]
            acc8 = acc8s[ib]
            # ---- phase 2: diagonal-block reduction + matching ----
            nc.vector.tensor_copy(out=c_sb[:, :], in_=psum_a[:, :])
            for q in range(nq):
                nc.tensor.matmul(
                    psg[:, :],
                    lhsT=ident[0:cdim, 8 * q : 8 * q + 8],
                    rhs=c_sb[:, 8 * q : 8 * q + 8],
                    start=(q == 0),
                    stop=(q == nq - 1),
                )
            nc.vector.tensor_copy(out=acc8[:, :], in_=psg[0:8, 0:8])

            if host_match:
                nc.sync.dma_start(
                    out=loss_t[ib * 8 : (ib + 1) * 8, :], in_=acc8[0:8, 0:8]
                )
                return

            # flatten Gram to one partition: BD[j, 8i+k] = acc8[j,k]*m[j,i,k]
            # (0.5 on the diagonal), then F[0, 8i+k] = sum_j BD[j, 8i+k]
            bd = small_pool.tile([8, 64], F32, tag=f"bd{ib}")
            nc.vector.tensor_mul(
                out=bd[0:8, :].rearrange("p (i k) -> p i k", k=8),
                in0=mask8s,
                in1=acc8[0:8, 0:8].unsqueeze(1).broadcast_to((8, 8, 8)),
            )
            psf = psumf_pool.tile([1, 72], F32, tag=f"psf{ib}")
            nc.tensor.matmul(
                psf[0:1, 0:64], lhsT=ones8, rhs=bd[0:8, :], start=True, stop=True
            )

            # s = 0.5*pn + 0.5*gn - cross  (= 0.5*d^2; monotone in d, so the
            # greedy matching runs on s; host computes sqrt(2*s))
            flat = small_pool.tile([1, 72], F32, tag=f"flat{ib}")
            nc.vector.tensor_copy(out=flat[0:1, 0:64], in_=psf[0:1, 0:64])
            g9 = flat[0:1, 0:72].rearrange("p (a b) -> p a b", b=9)
            pnh = g9[:, 0:4, 0:1].broadcast_to((1, 4, 4))
            gnh = g9[:, 4:8, 0:1].transpose([0, 2, 1]).broadcast_to((1, 4, 4))
            cross = flat[0:1, 0:64].rearrange("p (a b) -> p a b", b=8)[:, 0:4, 4:8]

            t16 = small_pool.tile([1, 16], F32, tag=f"t{ib}")
            t16v = t16[0:1, :].rearrange("p (a b) -> p a b", b=4)
            s16 = small_pool.tile([1, 16], F32, tag=f"s{ib}")
            s16v = s16[0:1, :].rearrange("p (a b) -> p a b", b=4)

            if safe:
                # baseline op classes only.  s = pn/2 + gn/2 - cross; per
                # greedy iteration: min -> fused (s<=min)*BIG mask ->
                # row/col maxima -> add the two penalty vectors into s.
                maskb = small_pool.tile([1, 16], F32, tag=f"mask{ib}")
                maskbv = maskb[0:1, :].rearrange("p (a b) -> p a b", b=4)
                rowm = small_pool.tile([1, 4], F32, tag=f"rowm{ib}")
                colm = small_pool.tile([1, 4], F32, tag=f"colm{ib}")
                nc.vector.tensor_add(out=s16v, in0=pnh, in1=gnh)
                nc.vector.tensor_sub(out=s16v, in0=s16v, in1=cross)
                for it in range(S):
                    slot = loss4[0:1, ib * S + it : ib * S + it + 1]
                    nc.vector.tensor_reduce(
                        out=slot,
                        in_=s16[:, :],
                        axis=mybir.AxisListType.X,
                        op=mybir.AluOpType.min,
                    )
                    if it == S - 1:
                        break
                    nc.vector.tensor_scalar(
                        out=maskb[:, :],
                        in0=s16[:, :],
                        scalar1=slot,
                        scalar2=BIG,
                        op0=mybir.AluOpType.is_le,
                        op1=mybir.AluOpType.mult,
                    )
                    nc.vector.tensor_reduce(
                        out=rowm[:, :],
                        in_=maskbv,
                        axis=mybir.AxisListType.X,
                        op=mybir.AluOpType.max,
                    )
                    nc.vector.tensor_reduce(
                        out=colm[:, :],
                        in_=maskbv.transpose([0, 2, 1]),
                        axis=mybir.AxisListType.X,
                        op=mybir.AluOpType.max,
                    )
                    nc.vector.tensor_add(
                        out=s16v,
                        in0=s16v,
                        in1=rowm[0:1, :].unsqueeze(2).broadcast_to((1, 4, 4)),
                    )
                    nc.vector.tensor_add(
                        out=s16v,
                        in0=s16v,
                        in1=colm[0:1, :].unsqueeze(1).broadcast_to((1, 4, 4)),
                    )
            else:
                nc.vector.tensor_add(out=t16v, in0=pnh, in1=gnh)
                cmp256 = small_pool.tile([1, 256], F32, tag=f"cmp{ib}")
                cmp256v = cmp256[0:1, :].rearrange("p (j k) -> p j k", k=16)
                pen = small_pool.tile([1, 16], F32, tag=f"pen{ib}")
                # s = t16 - cross, and its min is greedy pick #0 -- fused
                nc.vector.tensor_tensor_reduce(
                    out=s16v,
                    in0=t16v,
                    in1=cross,
                    scale=1.0,
                    scalar=FMAX,
                    op0=mybir.AluOpType.subtract,
                    op1=mybir.AluOpType.min,
                    accum_out=loss4[0:1, ib * S : ib * S + 1],
                    opt_aps=False,
                )
                for it in range(1, S):
                    prev = loss4[0:1, ib * S + it - 1 : ib * S + it]
                    nc.vector.scalar_tensor_tensor(
                        out=cmp256v,
                        in0=s16[0:1, :].unsqueeze(1).broadcast_to((1, 16, 16)),
                        scalar=prev,
                        in1=tbl16,
                        op0=mybir.AluOpType.is_le,
                        op1=mybir.AluOpType.mult,
                    )
                    nc.vector.tensor_reduce(
                        out=pen[:, :],
                        in_=cmp256v,
                        axis=mybir.AxisListType.X,
                        op=mybir.AluOpType.max,
                    )
                    nc.vector.tensor_tensor_reduce(
                        out=s16[:, :],
                        in0=s16[:, :],
                        in1=pen[:, :],
                        scale=1.0,
                        scalar=FMAX,
                        op0=mybir.AluOpType.add,
                        op1=mybir.AluOpType.min,
                        accum_out=loss4[0:1, ib * S + it : ib * S + it + 1],
                    )

            nc.sync.dma_start(
                out=loss_t[0:1, ib * S : (ib + 1) * S],
                in_=loss4[0:1, ib * S : (ib + 1) * S],
            )



        # Emission order: all of batch 0, then batch 1's first tile, then
        # batch 0's phase 2 -- so the PE chews batch 1's tile-0 matmuls
        # while the DVE copies batch 0's Gram out of PSUM, instead of
        # stalling at the selector matmuls' dependency.
        for tb in range(ntiles):
            emit_tile(0, tb)
        emit_tile(1, 0)
        emit_phase2(0)
        for tb in range(1, ntiles):
            emit_tile(1, tb)
        emit_phase2(1)

_NC_CACHE: dict = {}


def _get_nc(variant: str = VARIANT):
    if variant not in _NC_CACHE:
        _NC_CACHE[variant] = build_nc(variant)
    return _NC_CACHE[variant]


def shard_inputs(preds: np.ndarray, gts: np.ndarray, variant: str = VARIANT):
    """Quantize to fp8 e4m3 and build the interleaved layout, sliced per
    core (b is outermost, so per-core slices are contiguous views)."""
    p8 = np.asarray(preds).astype(NP_FP8).view(np.uint8)  # [S, T, B, D]
    g8 = np.asarray(gts).astype(NP_FP8).view(np.uint8)    # [S, B, T, D]
    if variant in ("fp8dr", "fp8drh", "fp8drsafe"):
        X = np.empty((B, 2, 128, 2, 64, 8, 8), np.uint8)  # b,tb,p,i,g,id,v
        X[..., 0:S] = p8.reshape(S, 2, 2, 128, B, 64, 8).transpose(
            4, 1, 3, 2, 5, 6, 0
        )
        X[..., S:] = g8.reshape(S, B, 2, 2, 128, 64, 8).transpose(
            1, 2, 4, 3, 5, 6, 0
        )
        X = X.reshape(B, 2, 128, 8192).view(NP_FP8)
    else:
        X = np.empty((B, 4, 128, 32, 16, 8), np.uint8)    # b,tb,p,g,i16,v
        X[..., 0:S] = p8.reshape(S, 4, 128, B, 32, 16).transpose(3, 1, 2, 4, 5, 0)
        X[..., S:] = g8.reshape(S, B, 4, 128, 32, 16).transpose(1, 2, 3, 4, 5, 0)
        X = X.reshape(B, 4, 128, 4096).view(NP_FP8)
    in_maps = []
    for c in range(N_CORES):
        b0 = c * NB
        in_maps.append({"xa": X[b0 : b0 + NB], "consts": CONSTS})
    return in_maps


kernel_last_results = None


def kernel(preds: np.ndarray, gts: np.ndarray) -> np.ndarray:
    global kernel_last_results
    nc = _get_nc()
    in_maps = shard_inputs(np.asarray(preds), np.asarray(gts))
    trace = os.environ.get("MINLOSS_TRACE", "1") == "1"
    try:
        res = run_bass_kernel_spmd(
            nc, in_maps, core_ids=list(range(N_CORES)), trace=trace
        )
    except Exception:
        if not trace:
            raise
        # profiling infrastructure may be unavailable; rerun without it
        res = run_bass_kernel_spmd(
            nc, in_maps, core_ids=list(range(N_CORES)), trace=False
        )
    kernel_last_results = res
    total = 0.0
    if VARIANT == "fp8drh":
        idx = np.arange(256)
        jj, kk = idx // 16, idx % 16
        tbl = np.where(
            (jj // 4 == kk // 4) | (jj % 4 == kk % 4), BIG, 0.0
        ).reshape(16, 16)
        for c in range(N_CORES):
            grams = np.asarray(res.results[c]["loss"], dtype=np.float64)
            for ib in range(NB):
                R = grams[ib * 8 : (ib + 1) * 8]
                pn = np.diag(R)[0:4]
                gn = np.diag(R)[4:8]
                s = (
                    0.5 * pn[:, None] + 0.5 * gn[None, :] - R[0:4, 4:8]
                ).reshape(-1)
                for _ in range(S):
                    m = s.min()
                    total += float(np.sqrt(max(2.0 * m, 0.0)))
                    mask = (s <= m).astype(np.float64)
                    s = s + (tbl * mask[None, :]).max(axis=1)
    else:
        for c in range(N_CORES):
            m2 = np.asarray(res.results[c]["loss"], dtype=np.float64)
            total += float(np.sqrt(np.maximum(2.0 * m2, 0.0)).sum())
    return np.array(total, dtype=np.float32)


# revision 29
# speedup vs baseline: 1.1044x; 1.1044x over previous
"""Trainium2 Bass kernel for nn_MinLoss_12343736009330.

Math: the reference loss is
    loss = sum_{b,s} || pf[b,s] - gf[b,match[b,s]] ||_2
where pf/gf are the per-(batch, source) flattened [L=T*D] signals, and match
is a greedy assignment on the 4x4 Euclidean cdist.  Since
    ||pf[s] - gf[m]||^2 = pn[s] + gn[m] - 2 <pf[s], gf[m]>,
the whole computation reduces to the per-batch 8x8 Gram matrix of the
8 vectors {pf[0..4], gf[0..4]} plus a tiny 4x4 greedy matching.

Sharding: batch axis (16) across 8 cores -> 2 batches/core.

v2 (default "fp8safe", ~41us vs the 69-72us fp32/bf16 baseline): inputs
are quantized to fp8 e4m3 on the host (measured loss error ~4e-4 vs the
2e-2 gate), quartering HBM traffic vs fp32.  ~22 junk matmuls on a
memset tile pre-ramp the PE p-state while the first chunk is in flight;
each tile then streams in column chunks alternating across the sync and
gpsimd DMA rings while plain fp8 matmuls (1 col/cycle, same rate as
bf16 but 1/4 the bytes) accumulate the interleaved Gram per batch.
Selector matmuls reduce the 8x8 diagonal blocks, one DVE op + one
matmul flatten the Gram to a single partition with 0.5 pre-scaled
norms, so s = 0.5*d^2 and the greedy matching runs on s using row/col
penalty vectors (cheaper than the 16x16 penalty-table form).  Per-batch
structure hides batch 0's reduction and matching under batch 1's DMA.
Host applies sqrt(2s) and sums across cores.

Measured on this setup: per-core DMA is throttled to ~280 GB/s
aggregate no matter how many rings are used; PE fp8 matmuls run ~56ns
per [128,128] instruction (DoubleRow perf mode executes but is NOT
faster on real HW: ~73ns); the fused DVE ops (tensor_tensor_reduce /
scalar_tensor_tensor) crashed NEFF execution and are kept only in the
non-default variants; the measured window carries a fixed ~7us
semaphore-teardown epilogue and ~3us of DMA spin-up that no kernel
structure avoids.
"""

import os
import sys

import numpy as np
import ml_dtypes

try:
    import concourse.bass as bass  # noqa: F401
except ImportError:
    sys.path.insert(0, "/opt/trn_rl_repo")

import concourse.bass as bass  # noqa: F811
import concourse.tile as tile
from concourse import bacc, mybir
from concourse.bass_utils import run_bass_kernel_spmd


def _install_ntff_hook_shim():
    """The bare agent image lacks ``antenv.axon_hooks``, so trace=True under
    axon would ImportError.  Recreate the module with the ctypes-based NTFF
    hook from trn_agent_boot (degrades to hook=None if unavailable)."""
    import types

    try:
        import antenv.axon_hooks  # noqa: F401

        return
    except ImportError:
        pass
    hook = None
    try:
        from trn_agent_boot.trn_boot import _ntff_profile_via_ctypes

        so_path = "/opt/axon/libaxon_pjrt.so"
        if os.path.exists(so_path):
            hook = _ntff_profile_via_ctypes(so_path)
    except Exception:
        hook = None
    import antenv

    mod = types.ModuleType("antenv.axon_hooks")
    mod.get_axon_ntff_profile_hook = lambda: hook  # type: ignore[attr-defined]

    def _set(h):
        nonlocal hook
        hook = h

    mod.set_axon_ntff_profile_hook = _set  # type: ignore[attr-defined]
    sys.modules["antenv.axon_hooks"] = mod
    antenv.axon_hooks = mod


_install_ntff_hook_shim()

F32 = mybir.dt.float32
FP8 = mybir.dt.float8e4
NP_FP8 = ml_dtypes.float8_e4m3

S, T, B, D = 4, 512, 16, 512
N_CORES = 8
NB = B // N_CORES          # batches per core
BIG = 1.0e30
FMAX = 3.0e38

# "fp8dr":   DoubleRow fp8 matmuls (2 cols/cycle), [64,64] Gram PSUM.
# "fp8":     plain fp8 matmuls, [128,128] Gram PSUM (fallback).
# "fp8drh":  fp8dr but the device returns the per-batch 8x8 Grams and the
#            4x4 greedy matching runs on the host with the final reduction.
# "fp8safe": plain fp8 matmuls + baseline-style DVE ops only (no fused
#            reduce ops, no Activation-ring DMA).
VARIANT = os.environ.get("MINLOSS_V", "fp8safe")


def _build_consts() -> np.ndarray:
    """Host-side constant block, DMA'd once: [128, 512] fp32.

    row 0, cols 0:256: penalty table TBL[j*16+k] = BIG if entries j and k
    of the flattened 4x4 dist matrix share a row or column.
    cols 256:384: 128x128 identity (diagonal-block selector matmuls; the
    fp8dr variant uses the top-left 64x64 of it).
    rows 0:8, cols 384:448: flatten mask M[j, 8i+k] = (i==j)*(0.5 if k==j
    else 1) -- the 0.5 pre-halves the norms so s = 0.5*d^2 comes out of
    the flatten matmul directly.
    rows 0:8, col 448: ones (flatten matmul stationary).
    """
    c = np.zeros((128, 512), np.float32)
    idx = np.arange(256)
    jj, kk = idx // 16, idx % 16
    c[0, 0:256] = np.where((jj // 4 == kk // 4) | (jj % 4 == kk % 4), BIG, 0.0)
    c[:, 256:384] = np.eye(128, dtype=np.float32)
    m = np.zeros((8, 8, 8), np.float32)
    for j in range(8):
        m[j, j, :] = 1.0
        m[j, j, j] = 0.5
    c[0:8, 384:448] = m.reshape(8, 64)
    c[0:8, 448] = 1.0
    return c


CONSTS = _build_consts()


def build_nc(variant: str = VARIANT):
    nc = bacc.Bacc(
        "TRN2",
        target_bir_lowering=False,
        debug=False,
        enable_asserts=True,
        num_devices=N_CORES,
    )
    if variant in ("fp8dr", "fp8drh", "fp8drsafe"):
        # xa[b, tb, p, i*4096 + g*64 + id*8 + v] = vec v at
        # t = 256*tb + 128*i + p, d = 8*g + id  (v 0..3 preds, 4..7 gts)
        xa_t = nc.dram_tensor("xa", [NB, 2, 128, 8192], FP8, kind="ExternalInput").ap()
    else:
        # xa[b, tb, p, g*128 + i16*8 + v] = vec v at t = 128*tb + p,
        # d = 16*g + i16
        xa_t = nc.dram_tensor("xa", [NB, 4, 128, 4096], FP8, kind="ExternalInput").ap()
    consts_t = nc.dram_tensor("consts", [128, 512], F32, kind="ExternalInput").ap()
    # the 8 greedy minima (0.5 * squared distances); host: sqrt(2x) + sum.
    # fp8drh instead returns the per-batch 8x8 Grams in "loss" [NB*8, 8].
    oshape = [NB * 8, 8] if variant in ("fp8drh", "fp8safeh") else [1, 2 * S]
    loss_t = nc.dram_tensor("loss", oshape, F32, kind="ExternalOutput").ap()

    with tile.TileContext(nc) as tc:
        _build_tile(tc, xa_t, consts_t, loss_t, variant)

    nc.compile()
    return nc


def _build_tile(tc, xa_t, consts_t, loss_t, variant):
    nc = tc.nc
    import contextlib

    dr = variant in ("fp8dr", "fp8drh", "fp8drsafe")
    host_match = variant in ("fp8drh", "fp8safeh")
    safe = variant in ("fp8safe", "fp8drsafe", "fp8safeh")
    ntiles = 2 if dr else 4            # DMA tiles per batch
    ngr = 64 if dr else 32             # matmul groups per tile
    cdim = 64 if dr else 128           # Gram PSUM dim
    nq = cdim // 8                     # selector matmuls per Gram

    ctx = contextlib.ExitStack()
    with ctx:
        b_pool = ctx.enter_context(tc.tile_pool(name="b", bufs=NB * ntiles))
        psum_pool = ctx.enter_context(tc.tile_pool(name="psum", bufs=1, space="PSUM"))
        psumf_pool = ctx.enter_context(tc.tile_pool(name="psumf", bufs=1, space="PSUM"))
        consts_pool = ctx.enter_context(tc.tile_pool(name="consts", bufs=1))
        small_pool = ctx.enter_context(tc.tile_pool(name="small", bufs=1))

        csb = consts_pool.tile([128, 512], F32)
        # consts ride the late-starting gpsimd ring, which also carries the
        # smallest data share
        nc.gpsimd.dma_start(out=csb[:, :], in_=consts_t[:, :])
        tbl16 = csb[0:1, 0:256].rearrange("p (j k) -> p j k", k=16)
        ident = csb[:, 256:384]
        mask8s = csb[0:8, 384:448].rearrange("p (i k) -> p i k", k=8)
        ones8 = csb[0:8, 448:449]

        # the 8 greedy minima (0.5 * squared dists)
        loss4 = small_pool.tile([1, 2 * S], F32, tag="loss4")

        if safe:
            rings = [nc.sync, nc.scalar, nc.gpsimd]
        else:
            rings = [nc.sync, nc.scalar]
        ring_cnt = [0]

        def next_ring():
            r = rings[ring_cnt[0] % len(rings)]
            ring_cnt[0] += 1
            return r

        def chunk_ring(ib, tb, ch, nch):
            return next_ring()

        psum_as = [
            psum_pool.tile([cdim, cdim], F32, tag=f"pA{i}", name=f"psum_a{i}")
            for i in range(NB)
        ]

        # Junk matmuls on a memset tile ramp the PE p-state (0.65 -> 2.4
        # GHz needs a few us of continuous execution) while the first data
        # chunk is still in flight.
        nwarm = int(os.environ.get("MINLOSS_WARM", "22"))
        if nwarm:
            warm_ps = psum_pool.tile([cdim, cdim], F32, tag="warmps")
            wsb = small_pool.tile([128, 128], FP8, tag="warm")
            nc.vector.memset(wsb[:, :], 1.0)
            for _ in range(nwarm):
                nc.tensor.matmul(
                    warm_ps[:, :],
                    lhsT=wsb[:, 0:cdim],
                    rhs=wsb[:, 0:cdim],
                    start=True,
                    stop=True,
                )

        psgs = [psumf_pool.tile([8, 8], F32, tag=f"psg{i}", name=f"psg{i}") for i in range(NB)]
        c_sbs = [
            small_pool.tile([cdim, cdim], F32, tag=f"c{i}", name=f"c_sb{i}")
            for i in range(NB)
        ]
        acc8s = [
            small_pool.tile([8, 8], F32, tag=f"acc{i}", name=f"acc8{i}")
            for i in range(NB)
        ]

        def emit_tile(ib, tb):
            psum_a = psum_as[ib]
            if True:
                last_tile = tb == ntiles - 1
                psum = psum_a
                if dr:
                    xtile = b_pool.tile([128, 8192], FP8)
                    xv = xtile[:, :].rearrange("p (i c) -> p i c", i=2)
                    iv = xa_t[ib, tb, :, :].rearrange("p (i c) -> p i c", i=2)
                else:
                    xtile = b_pool.tile([128, 4096], FP8)
                    xv = xtile[:, :]
                    iv = xa_t[ib, tb, :, :]

                def emit_mm(g, psum=psum, xv=xv, tb=tb, last_tile=last_tile):
                    first = tb == 0 and g == 0
                    last = last_tile and g == ngr - 1
                    if dr:
                        op = xv[:, :, g * 64 : (g + 1) * 64]
                        nc.tensor.matmul(
                            psum[:, :],
                            lhsT=op,
                            rhs=op,
                            start=first,
                            stop=last,
                            perf_mode=mybir.MatmulPerfMode.DoubleRow,
                        )
                    else:
                        op = xv[:, g * 128 : (g + 1) * 128]
                        nc.tensor.matmul(
                            psum[:, :], lhsT=op, rhs=op, start=first, stop=last
                        )

                if True:
                    # column chunks: matmuls on chunk k stream while chunk
                    # k+1 is still in flight; finer chunks on the last tile
                    # so its matmuls overlap the DMA tail
                    nch = 4 if (last_tile or (ib == 0 and tb == 0)) else 2
                    gpc = ngr // nch
                    for ch in range(nch):
                        g0, g1 = ch * gpc, (ch + 1) * gpc
                        if dr:
                            osl = xv[:, :, g0 * 64 : g1 * 64]
                            isl = iv[:, :, g0 * 64 : g1 * 64]
                        else:
                            osl = xv[:, g0 * 128 : g1 * 128]
                            isl = iv[:, g0 * 128 : g1 * 128]
                        chunk_ring(ib, tb, ch, nch).dma_start(out=osl, in_=isl)
                        for g in range(g0, g1):
                            emit_mm(g)

        def emit_phase2(ib):
            psum_a = psum_as[ib]
            psg = psgs[ib]
            c_sb = c_sbs[ib]
            acc8 = acc8s[ib]
            # ---- phase 2: diagonal-block reduction + matching ----
            nc.vector.tensor_copy(out=c_sb[:, :], in_=psum_a[:, :])
            for q in range(nq):
                nc.tensor.matmul(
                    psg[:, :],
                    lhsT=ident[0:cdim, 8 * q : 8 * q + 8],
                    rhs=c_sb[:, 8 * q : 8 * q + 8],
                    start=(q == 0),
                    stop=(q == nq - 1),
                )
            nc.vector.tensor_copy(out=acc8[:, :], in_=psg[0:8, 0:8])

            if host_match:
                nc.sync.dma_start(
                    out=loss_t[ib * 8 : (ib + 1) * 8, :], in_=acc8[0:8, 0:8]
                )
                return

            # flatten Gram to one partition: BD[j, 8i+k] = acc8[j,k]*m[j,i,k]
            # (0.5 on the diagonal), then F[0, 8i+k] = sum_j BD[j, 8i+k]
            bd = small_pool.tile([8, 64], F32, tag=f"bd{ib}")
            nc.vector.tensor_mul(
                out=bd[0:8, :].rearrange("p (i k) -> p i k", k=8),
                in0=mask8s,
                in1=acc8[0:8, 0:8].unsqueeze(1).broadcast_to((8, 8, 8)),
            )
            psf = psumf_pool.tile([1, 72], F32, tag=f"psf{ib}")
            nc.tensor.matmul(
                psf[0:1, 0:64], lhsT=ones8, rhs=bd[0:8, :], start=True, stop=True
            )

            # s = 0.5*pn + 0.5*gn - cross  (= 0.5*d^2; monotone in d, so the
            # greedy matching runs on s; host computes sqrt(2*s))
            flat = small_pool.tile([1, 72], F32, tag=f"flat{ib}")
            nc.vector.tensor_copy(out=flat[0:1, 0:64], in_=psf[0:1, 0:64])
            g9 = flat[0:1, 0:72].rearrange("p (a b) -> p a b", b=9)
            pnh = g9[:, 0:4, 0:1].broadcast_to((1, 4, 4))
            gnh = g9[:, 4:8, 0:1].transpose([0, 2, 1]).broadcast_to((1, 4, 4))
            cross = flat[0:1, 0:64].rearrange("p (a b) -> p a b", b=8)[:, 0:4, 4:8]

            t16 = small_pool.tile([1, 16], F32, tag=f"t{ib}")
            t16v = t16[0:1, :].rearrange("p (a b) -> p a b", b=4)
            s16 = small_pool.tile([1, 16], F32, tag=f"s{ib}")
            s16v = s16[0:1, :].rearrange("p (a b) -> p a b", b=4)

            if safe:
                # baseline op classes only.  s = pn/2 + gn/2 - cross; per
                # greedy iteration: min -> fused (s<=min)*BIG mask ->
                # row/col maxima -> add the two penalty vectors into s.
                maskb = small_pool.tile([1, 16], F32, tag=f"mask{ib}")
                maskbv = maskb[0:1, :].rearrange("p (a b) -> p a b", b=4)
                rowm = small_pool.tile([1, 4], F32, tag=f"rowm{ib}")
                colm = small_pool.tile([1, 4], F32, tag=f"colm{ib}")
                nc.vector.tensor_add(out=s16v, in0=pnh, in1=gnh)
                nc.vector.tensor_sub(out=s16v, in0=s16v, in1=cross)
                for it in range(S):
                    slot = loss4[0:1, ib * S + it : ib * S + it + 1]
                    nc.vector.tensor_reduce(
                        out=slot,
                        in_=s16[:, :],
                        axis=mybir.AxisListType.X,
                        op=mybir.AluOpType.min,
                    )
                    if it == S - 1:
                        break
                    nc.vector.tensor_scalar(
                        out=maskb[:, :],
                        in0=s16[:, :],
                        scalar1=slot,
                        scalar2=BIG,
                        op0=mybir.AluOpType.is_le,
                        op1=mybir.AluOpType.mult,
                    )
                    nc.vector.tensor_reduce(
                        out=rowm[:, :],
                        in_=maskbv,
                        axis=mybir.AxisListType.X,
                        op=mybir.AluOpType.max,
                    )
                    nc.vector.tensor_reduce(
                        out=colm[:, :],
                        in_=maskbv.transpose([0, 2, 1]),
                        axis=mybir.AxisListType.X,
                        op=mybir.AluOpType.max,
                    )
                    nc.vector.tensor_add(
                        out=s16v,
                        in0=s16v,
                        in1=rowm[0:1, :].unsqueeze(2).broadcast_to((1, 4, 4)),
                    )
                    nc.vector.tensor_add(
                        out=s16v,
                        in0=s16v,
                        in1=colm[0:1, :].unsqueeze(1).broadcast_to((1, 4, 4)),
                    )
            else:
                nc.vector.tensor_add(out=t16v, in0=pnh, in1=gnh)
                cmp256 = small_pool.tile([1, 256], F32, tag=f"cmp{ib}")
                cmp256v = cmp256[0:1, :].rearrange("p (j k) -> p j k", k=16)
                pen = small_pool.tile([1, 16], F32, tag=f"pen{ib}")
                # s = t16 - cross, and its min is greedy pick #0 -- fused
                nc.vector.tensor_tensor_reduce(
                    out=s16v,
                    in0=t16v,
                    in1=cross,
                    scale=1.0,
                    scalar=FMAX,
                    op0=mybir.AluOpType.subtract,
                    op1=mybir.AluOpType.min,
                    accum_out=loss4[0:1, ib * S : ib * S + 1],
                    opt_aps=False,
                )
                for it in range(1, S):
                    prev = loss4[0:1, ib * S + it - 1 : ib * S + it]
                    nc.vector.scalar_tensor_tensor(
                        out=cmp256v,
                        in0=s16[0:1, :].unsqueeze(1).broadcast_to((1, 16, 16)),
                        scalar=prev,
                        in1=tbl16,
                        op0=mybir.AluOpType.is_le,
                        op1=mybir.AluOpType.mult,
                    )
                    nc.vector.tensor_reduce(
                        out=pen[:, :],
                        in_=cmp256v,
                        axis=mybir.AxisListType.X,
                        op=mybir.AluOpType.max,
                    )
                    nc.vector.tensor_tensor_reduce(
                        out=s16[:, :],
                        in0=s16[:, :],
                        in1=pen[:, :],
                        scale=1.0,
                        scalar=FMAX,
                        op0=mybir.AluOpType.add,
                        op1=mybir.AluOpType.min,
                        accum_out=loss4[0:1, ib * S + it : ib * S + it + 1],
                    )

            nc.sync.dma_start(
                out=loss_t[0:1, ib * S : (ib + 1) * S],
                in_=loss4[0:1, ib * S : (ib + 1) * S],
            )



        # Emission order: all of batch 0, then batch 1's first tile, then
        # batch 0's phase 2 -- so the PE chews batch 1's tile-0 matmuls
        # while the DVE copies batch 0's Gram out of PSUM, instead of
        # stalling at the selector matmuls' dependency.
        for tb in range(ntiles):
            emit_tile(0, tb)
        emit_tile(1, 0)
        emit_phase2(0)
        for tb in range(1, ntiles):
            emit_tile(1, tb)
        emit_phase2(1)

_NC_CACHE: dict = {}


def _get_nc(variant: str = VARIANT):
    if variant not in _NC_CACHE:
        _NC_CACHE[variant] = build_nc(variant)
    return _NC_CACHE[variant]


def shard_inputs(preds: np.ndarray, gts: np.ndarray, variant: str = VARIANT):
    """Quantize to fp8 e4m3 and build the interleaved layout, sliced per
    core (b is outermost, so per-core slices are contiguous views)."""
    p8 = np.asarray(preds).astype(NP_FP8).view(np.uint8)  # [S, T, B, D]
    g8 = np.asarray(gts).astype(NP_FP8).view(np.uint8)    # [S, B, T, D]
    if variant in ("fp8dr", "fp8drh", "fp8drsafe"):
        X = np.empty((B, 2, 128, 2, 64, 8, 8), np.uint8)  # b,tb,p,i,g,id,v
        X[..., 0:S] = p8.reshape(S, 2, 2, 128, B, 64, 8).transpose(
            4, 1, 3, 2, 5, 6, 0
        )
        X[..., S:] = g8.reshape(S, B, 2, 2, 128, 64, 8).transpose(
            1, 2, 4, 3, 5, 6, 0
        )
        X = X.reshape(B, 2, 128, 8192).view(NP_FP8)
    else:
        X = np.empty((B, 4, 128, 32, 16, 8), np.uint8)    # b,tb,p,g,i16,v
        X[..., 0:S] = p8.reshape(S, 4, 128, B, 32, 16).transpose(3, 1, 2, 4, 5, 0)
        X[..., S:] = g8.reshape(S, B, 4, 128, 32, 16).transpose(1, 2, 3, 4, 5, 0)
        X = X.reshape(B, 4, 128, 4096).view(NP_FP8)
    in_maps = []
    for c in range(N_CORES):
        b0 = c * NB
        in_maps.append({"xa": X[b0 : b0 + NB], "consts": CONSTS})
    return in_maps


kernel_last_results = None


def kernel(preds: np.ndarray, gts: np.ndarray) -> np.ndarray:
    global kernel_last_results
    nc = _get_nc()
    in_maps = shard_inputs(np.asarray(preds), np.asarray(gts))
    trace = os.environ.get("MINLOSS_TRACE", "1") == "1"
    try:
        res = run_bass_kernel_spmd(
            nc, in_maps, core_ids=list(range(N_CORES)), trace=trace
        )
    except Exception:
        if not trace:
            raise
        # profiling infrastructure may be unavailable; rerun without it
        res = run_bass_kernel_spmd(
            nc, in_maps, core_ids=list(range(N_CORES)), trace=False
        )
    kernel_last_results = res
    total = 0.0
    if VARIANT in ("fp8drh", "fp8safeh"):
        idx = np.arange(256)
        jj, kk = idx // 16, idx % 16
        tbl = np.where(
            (jj // 4 == kk // 4) | (jj % 4 == kk % 4), BIG, 0.0
        ).reshape(16, 16)
        for c in range(N_CORES):
            grams = np.asarray(res.results[c]["loss"], dtype=np.float64)
            for ib in range(NB):
                R = grams[ib * 8 : (ib + 1) * 8]
                pn = np.diag(R)[0:4]
                gn = np.diag(R)[4:8]
                s = (
                    0.5 * pn[:, None] + 0.5 * gn[None, :] - R[0:4, 4:8]
                ).reshape(-1)
                for _ in range(S):
                    m = s.min()
                    total += float(np.sqrt(max(2.0 * m, 0.0)))
                    mask = (s <= m).astype(np.float64)
                    s = s + (tbl * mask[None, :]).max(axis=1)
    else:
        for c in range(N_CORES):
            m2 = np.asarray(res.results[c]["loss"], dtype=np.float64)
            total += float(np.sqrt(np.maximum(2.0 * m2, 0.0)).sum())
    return np.array(total, dtype=np.float32)
